# revision 2
# baseline (speedup 1.0000x reference)
"""CRZ-ring fused diagonal phase rotation on 8 Trainium2 NeuronCores.

Computation (reference):
    p[d]  = 0.5 * sum_i bits[d,i] * (2*bits[d,(i+1)%14] - 1) * theta[i]
    out_r = state_real * cos(p) - state_imag * sin(p)
    out_i = state_real * sin(p) + state_imag * cos(p)
    out   = stack([out_r, out_i], axis=-1)          # [B, D, 2] f32

Strategy:
  - cos(p)/sin(p) ([D] vectors) are computed on the host (tiny: 16K elems).
  - Batch dim (2048) is sharded across 8 cores (256 rows each).
  - Per core, per chunk of C=2048 columns: cos/sin rows are broadcast to
    [128, C] tiles via a K=1 PE matmul (ones ⊗ row) + ScalarE PSUM→SBUF copy.
    VectorE does the 4 products, GPSIMD does the 2 interleaving add/subs
    writing the [128, 2C] interleaved output tile, HWDGE DMA does all I/O.
"""

import numpy as np

B = 2048
D = 16384
N_WIRES = 14
N_CORES = 8
ROWS = B // N_CORES      # 256 batch rows per core
RG = 128                 # rows per partition group
C = 2048                 # d-chunk per tile
N_CHUNK = D // C
MM_N = 512               # matmul moving free dim (one PSUM bank)

_CACHED_NC = None


def _phase_cos_sin(theta: np.ndarray):
    """Host-side computation of cos/sin of the ring phase (f64 -> f32)."""
    idx = np.arange(D, dtype=np.int64)
    shifts = (N_WIRES - 1) - np.arange(N_WIRES)
    bits = ((idx[:, None] >> shifts[None, :]) & 1).astype(np.float64)
    tgt_sign = 2.0 * np.roll(bits, -1, axis=1) - 1.0
    p = 0.5 * ((bits * tgt_sign) @ theta.astype(np.float64))
    return np.cos(p).astype(np.float32), np.sin(p).astype(np.float32)


def _split_multiwaits(nc):
    """Walrus in this container supports at most one sync-wait per
    instruction; hoist extra Tile-assigned waits onto single-wait NoOps."""
    import concourse.mybir as mybir

    for f in nc.m.functions:
        new_blocks = []
        for bb in f.blocks:
            insts = list(bb.instructions)
            if not any(
                i.sync_info is not None and len(i.sync_info.on_wait) > 1
                for i in insts
            ):
                new_blocks.append(bb)
                continue
            out = []
            for i in insts:
                si = i.sync_info
                if si is not None and len(si.on_wait) > 1:
                    waits = list(si.on_wait)
                    for k, w in enumerate(waits[:-1]):
                        out.append(
                            mybir.InstNoOp(
                                name=f"{i.name}-sw{k}",
                                engine=i.engine,
                                bass_nofuse=True,
                                sync_info=mybir.SyncInfo(on_wait=[w], on_update=[]),
                            )
                        )
                    i.sync_info = mybir.SyncInfo(
                        on_wait=[waits[-1]], on_update=list(si.on_update)
                    )
                out.append(i)
            new_blocks.append(mybir.BasicBlock(name=bb.name, instructions=out))
        f.blocks = new_blocks


def _build_nc():
    import concourse.bass as bass
    import concourse.mybir as mybir
    from concourse.tile import TileContext

    nc = bass.Bass()
    f32 = mybir.dt.float32

    sr_d = nc.declare_dram_parameter("state_real", [ROWS, D], f32, isOutput=False)
    si_d = nc.declare_dram_parameter("state_imag", [ROWS, D], f32, isOutput=False)
    cs_d = nc.declare_dram_parameter("cs", [2, D], f32, isOutput=False)
    ones_d = nc.declare_dram_parameter("ones", [1, 128], f32, isOutput=False)
    out_d = nc.declare_dram_parameter("out", [ROWS, 2 * D], f32, isOutput=True)

    with TileContext(nc) as tc:
        with (
            tc.tile_pool(name="const", bufs=1) as const_pool,
            tc.tile_pool(name="coef", bufs=2) as coef_pool,
            tc.tile_pool(name="io", bufs=3) as io_pool,
            tc.tile_pool(name="tmp", bufs=3) as tmp_pool,
            tc.tile_pool(name="psum", bufs=2, space="PSUM") as psum_pool,
        ):
            ones_t = const_pool.tile([1, 128], f32)
            nc.sync.dma_start(out=ones_t, in_=ones_d[:, :])

            for ci in range(N_CHUNK):
                d0 = ci * C
                c_t = coef_pool.tile([1, C], f32, tag="c_row")
                s_t = coef_pool.tile([1, C], f32, tag="s_row")
                nc.sync.dma_start(out=c_t, in_=cs_d[0:1, d0 : d0 + C])
                nc.sync.dma_start(out=s_t, in_=cs_d[1:2, d0 : d0 + C])
                cb = coef_pool.tile([128, C], f32, tag="cb")
                sb = coef_pool.tile([128, C], f32, tag="sb")
                for j in range(0, C, MM_N):
                    pc = psum_pool.tile([128, MM_N], f32, tag="pc")
                    nc.tensor.matmul(
                        pc, ones_t, c_t[:, j : j + MM_N], start=True, stop=True
                    )
                    nc.scalar.copy(out=cb[:, j : j + MM_N], in_=pc)
                    ps = psum_pool.tile([128, MM_N], f32, tag="ps")
                    nc.tensor.matmul(
                        ps, ones_t, s_t[:, j : j + MM_N], start=True, stop=True
                    )
                    nc.scalar.copy(out=sb[:, j : j + MM_N], in_=ps)

                for rg in range(ROWS // RG):
                    r0 = rg * RG
                    sr_t = io_pool.tile([RG, C], f32, tag="sr")
                    si_t = io_pool.tile([RG, C], f32, tag="si")
                    nc.sync.dma_start(out=sr_t, in_=sr_d[r0 : r0 + RG, d0 : d0 + C])
                    nc.sync.dma_start(out=si_t, in_=si_d[r0 : r0 + RG, d0 : d0 + C])

                    out_t = io_pool.tile([RG, 2 * C], f32, tag="out", bufs=2)
                    m1 = tmp_pool.tile([RG, C], f32, tag="m1")
                    m2 = tmp_pool.tile([RG, C], f32, tag="m2")
                    nc.vector.tensor_mul(out=m1, in0=sr_t, in1=cb)
                    nc.vector.tensor_mul(out=m2, in0=si_t, in1=sb)
                    nc.gpsimd.tensor_sub(
                        out=out_t[:, 0 : 2 * C : 2], in0=m1, in1=m2
                    )
                    m3 = tmp_pool.tile([RG, C], f32, tag="m1")
                    m4 = tmp_pool.tile([RG, C], f32, tag="m2")
                    nc.vector.tensor_mul(out=m3, in0=sr_t, in1=sb)
                    nc.vector.tensor_mul(out=m4, in0=si_t, in1=cb)
                    nc.gpsimd.tensor_add(
                        out=out_t[:, 1 : 2 * C : 2], in0=m3, in1=m4
                    )
                    nc.sync.dma_start(
                        out=out_d[r0 : r0 + RG, 2 * d0 : 2 * d0 + 2 * C], in_=out_t
                    )

    _split_multiwaits(nc)
    return nc


def _get_nc():
    global _CACHED_NC
    if _CACHED_NC is None:
        _CACHED_NC = _build_nc()
    return _CACHED_NC


def _make_in_maps(state_real, state_imag, theta):
    state_real = np.ascontiguousarray(np.asarray(state_real, dtype=np.float32))
    state_imag = np.ascontiguousarray(np.asarray(state_imag, dtype=np.float32))
    theta = np.asarray(theta, dtype=np.float32)
    c, s = _phase_cos_sin(theta)
    cs = np.ascontiguousarray(np.stack([c, s], axis=0))
    ones = np.ones((1, 128), dtype=np.float32)
    in_maps = []
    for k in range(N_CORES):
        r0 = k * ROWS
        in_maps.append(
            {
                "state_real": state_real[r0 : r0 + ROWS],
                "state_imag": state_imag[r0 : r0 + ROWS],
                "cs": cs,
                "ones": ones,
            }
        )
    return in_maps


def kernel(state_real, state_imag, theta):
    from concourse.bass_utils import run_bass_kernel_spmd

    nc = _get_nc()
    in_maps = _make_in_maps(state_real, state_imag, theta)
    res = run_bass_kernel_spmd(nc, in_maps, list(range(N_CORES)))
    out = np.empty((B, D, 2), dtype=np.float32)
    for k in range(N_CORES):
        out[k * ROWS : (k + 1) * ROWS] = res.results[k]["out"].reshape(ROWS, D, 2)
    return out


# revision 4
# speedup vs baseline: 150.6591x; 150.6591x over previous
"""CRZ-ring fused diagonal phase rotation on 8 Trainium2 NeuronCores.

Computation (reference):
    p[d]  = 0.5 * sum_i bits[d,i] * (2*bits[d,(i+1)%14] - 1) * theta[i]
    out_r = state_real * cos(p) - state_imag * sin(p)
    out_i = state_real * sin(p) + state_imag * cos(p)
    out   = stack([out_r, out_i], axis=-1)          # [B, D, 2] f32

Strategy:
  - cos(p)/sin(p) ([D] vectors) are computed on the host (tiny: 16K elems).
  - Batch dim (2048) is sharded across 8 cores (256 rows each).
  - Per core, per chunk of C=2048 columns: cos/sin rows are broadcast to
    [128, C] tiles via a K=1 PE matmul (ones ⊗ row) + ScalarE PSUM→SBUF copy.
    VectorE does the 4 products, GPSIMD does the 2 interleaving add/subs
    writing the [128, 2C] interleaved output tile, HWDGE DMA does all I/O.
"""

import numpy as np

B = 2048
D = 16384
N_WIRES = 14
N_CORES = 8
ROWS = B // N_CORES      # 256 batch rows per core
RG = 128                 # rows per partition group
C = 2048                 # d-chunk per tile
N_CHUNK = D // C
MM_N = 512               # matmul moving free dim (one PSUM bank)

_CACHED_NC = None


def _phase_cos_sin(theta: np.ndarray):
    """Host-side computation of cos/sin of the ring phase (f64 -> f32)."""
    idx = np.arange(D, dtype=np.int64)
    shifts = (N_WIRES - 1) - np.arange(N_WIRES)
    bits = ((idx[:, None] >> shifts[None, :]) & 1).astype(np.float64)
    tgt_sign = 2.0 * np.roll(bits, -1, axis=1) - 1.0
    p = 0.5 * ((bits * tgt_sign) @ theta.astype(np.float64))
    return np.cos(p).astype(np.float32), np.sin(p).astype(np.float32)


def _split_multiwaits(nc):
    """Walrus in this container supports at most one sync-wait per
    instruction; hoist extra Tile-assigned waits onto single-wait NoOps."""
    import concourse.mybir as mybir

    for f in nc.m.functions:
        new_blocks = []
        for bb in f.blocks:
            insts = list(bb.instructions)
            if not any(
                i.sync_info is not None and len(i.sync_info.on_wait) > 1
                for i in insts
            ):
                new_blocks.append(bb)
                continue
            out = []
            for i in insts:
                si = i.sync_info
                if si is not None and len(si.on_wait) > 1:
                    waits = list(si.on_wait)
                    for k, w in enumerate(waits[:-1]):
                        out.append(
                            mybir.InstNoOp(
                                name=f"{i.name}-sw{k}",
                                engine=i.engine,
                                bass_nofuse=True,
                                sync_info=mybir.SyncInfo(on_wait=[w], on_update=[]),
                            )
                        )
                    i.sync_info = mybir.SyncInfo(
                        on_wait=[waits[-1]], on_update=list(si.on_update)
                    )
                out.append(i)
            new_blocks.append(mybir.BasicBlock(name=bb.name, instructions=out))
        f.blocks = new_blocks


def _build_nc(loop_n=None):
    """Build the per-core Bass program.

    loop_n: if set, wrap the whole body in a runtime For_i loop executing it
    loop_n times (benchmarking only — output is idempotent).
    """
    import contextlib

    import concourse.bass as bass
    import concourse.mybir as mybir
    from concourse.tile import TileContext

    nc = bass.Bass()
    f32 = mybir.dt.float32

    sr_d = nc.declare_dram_parameter("state_real", [ROWS, D], f32, isOutput=False)
    si_d = nc.declare_dram_parameter("state_imag", [ROWS, D], f32, isOutput=False)
    cs_d = nc.declare_dram_parameter("cs", [2, D], f32, isOutput=False)
    ones_d = nc.declare_dram_parameter("ones", [1, 128], f32, isOutput=False)
    out_d = nc.declare_dram_parameter("out", [ROWS, 2 * D], f32, isOutput=True)

    with TileContext(nc) as tc:
        with (
            tc.tile_pool(name="const", bufs=1) as const_pool,
            tc.tile_pool(name="coef", bufs=2) as coef_pool,
            tc.tile_pool(name="io", bufs=3) as io_pool,
            tc.tile_pool(name="tmp", bufs=3) as tmp_pool,
            tc.tile_pool(name="psum", bufs=2, space="PSUM") as psum_pool,
        ):
            ones_t = const_pool.tile([1, 128], f32)
            nc.sync.dma_start(out=ones_t, in_=ones_d[:, :])

            loop_cm = (
                tc.For_i(0, loop_n, 1) if loop_n else contextlib.nullcontext()
            )
            with loop_cm:
                _emit_body(nc, tc, coef_pool, io_pool, tmp_pool, psum_pool,
                           ones_t, sr_d, si_d, cs_d, out_d, f32)

    _split_multiwaits(nc)
    return nc


def _emit_body(nc, tc, coef_pool, io_pool, tmp_pool, psum_pool,
               ones_t, sr_d, si_d, cs_d, out_d, f32):
            for ci in range(N_CHUNK):
                d0 = ci * C
                c_t = coef_pool.tile([1, C], f32, tag="c_row")
                s_t = coef_pool.tile([1, C], f32, tag="s_row")
                nc.sync.dma_start(out=c_t, in_=cs_d[0:1, d0 : d0 + C])
                nc.sync.dma_start(out=s_t, in_=cs_d[1:2, d0 : d0 + C])
                cb = coef_pool.tile([128, C], f32, tag="cb")
                sb = coef_pool.tile([128, C], f32, tag="sb")
                for j in range(0, C, MM_N):
                    pc = psum_pool.tile([128, MM_N], f32, tag="pc")
                    nc.tensor.matmul(
                        pc, ones_t, c_t[:, j : j + MM_N], start=True, stop=True
                    )
                    nc.scalar.copy(out=cb[:, j : j + MM_N], in_=pc)
                    ps = psum_pool.tile([128, MM_N], f32, tag="ps")
                    nc.tensor.matmul(
                        ps, ones_t, s_t[:, j : j + MM_N], start=True, stop=True
                    )
                    nc.scalar.copy(out=sb[:, j : j + MM_N], in_=ps)

                for rg in range(ROWS // RG):
                    r0 = rg * RG
                    sr_t = io_pool.tile([RG, C], f32, tag="sr")
                    si_t = io_pool.tile([RG, C], f32, tag="si")
                    nc.sync.dma_start(out=sr_t, in_=sr_d[r0 : r0 + RG, d0 : d0 + C])
                    nc.sync.dma_start(out=si_t, in_=si_d[r0 : r0 + RG, d0 : d0 + C])

                    out_t = io_pool.tile([RG, 2 * C], f32, tag="out", bufs=2)
                    m1 = tmp_pool.tile([RG, C], f32, tag="m1")
                    m2 = tmp_pool.tile([RG, C], f32, tag="m2")
                    nc.vector.tensor_mul(out=m1, in0=sr_t, in1=cb)
                    nc.vector.tensor_mul(out=m2, in0=si_t, in1=sb)
                    nc.gpsimd.tensor_sub(
                        out=out_t[:, 0 : 2 * C : 2], in0=m1, in1=m2
                    )
                    m3 = tmp_pool.tile([RG, C], f32, tag="m1")
                    m4 = tmp_pool.tile([RG, C], f32, tag="m2")
                    nc.vector.tensor_mul(out=m3, in0=sr_t, in1=sb)
                    nc.vector.tensor_mul(out=m4, in0=si_t, in1=cb)
                    nc.gpsimd.tensor_add(
                        out=out_t[:, 1 : 2 * C : 2], in0=m3, in1=m4
                    )
                    nc.sync.dma_start(
                        out=out_d[r0 : r0 + RG, 2 * d0 : 2 * d0 + 2 * C], in_=out_t
                    )


def _get_nc():
    global _CACHED_NC
    if _CACHED_NC is None:
        _CACHED_NC = _build_nc()
    return _CACHED_NC


def _make_in_maps(state_real, state_imag, theta):
    state_real = np.ascontiguousarray(np.asarray(state_real, dtype=np.float32))
    state_imag = np.ascontiguousarray(np.asarray(state_imag, dtype=np.float32))
    theta = np.asarray(theta, dtype=np.float32)
    c, s = _phase_cos_sin(theta)
    cs = np.ascontiguousarray(np.stack([c, s], axis=0))
    ones = np.ones((1, 128), dtype=np.float32)
    in_maps = []
    for k in range(N_CORES):
        r0 = k * ROWS
        in_maps.append(
            {
                "state_real": state_real[r0 : r0 + ROWS],
                "state_imag": state_imag[r0 : r0 + ROWS],
                "cs": cs,
                "ones": ones,
            }
        )
    return in_maps


def kernel(state_real, state_imag, theta):
    from concourse.bass_utils import run_bass_kernel_spmd

    nc = _get_nc()
    in_maps = _make_in_maps(state_real, state_imag, theta)
    res = run_bass_kernel_spmd(nc, in_maps, list(range(N_CORES)))
    out = np.empty((B, D, 2), dtype=np.float32)
    for k in range(N_CORES):
        out[k * ROWS : (k + 1) * ROWS] = res.results[k]["out"].reshape(ROWS, D, 2)
    return out


# revision 21
# speedup vs baseline: 319.8088x; 2.1227x over previous
"""CRZ-ring fused diagonal phase rotation on 8 Trainium2 NeuronCores.

Computation (reference):
    p[d]  = 0.5 * sum_i bits[d,i] * (2*bits[d,(i+1)%14] - 1) * theta[i]
    out_r = state_real * cos(p) - state_imag * sin(p)
    out_i = state_real * sin(p) + state_imag * cos(p)
    out   = stack([out_r, out_i], axis=-1)          # [B, D, 2] f32

Strategy:
  - cos(p)/sin(p) ([D] vectors) are computed on the host (tiny: 16K elems).
  - Batch dim (2048) is sharded across 8 cores (256 rows each).
  - Per core, per chunk of C=2048 columns: cos/sin rows are broadcast to
    [128, C] tiles via a K=1 PE matmul (ones ⊗ row) + ScalarE PSUM→SBUF copy.
    VectorE does the 4 products, GPSIMD does the 2 interleaving add/subs
    writing the [128, 2C] interleaved output tile, HWDGE DMA does all I/O.
"""

import numpy as np

B = 2048
D = 16384
N_WIRES = 14
N_CORES = 8
ROWS = B // N_CORES      # 256 batch rows per core
RG = 128                 # rows per partition group
C = 2048                 # d-chunk per tile
N_CHUNK = D // C
MM_N = 512               # matmul moving free dim (one PSUM bank)

_CACHED_NC = None

# Engine-split variant for the two interleaving add/subs (perf tuning):
#   "pool2"  - both on GPSIMD
#   "dve2"   - both on VectorE
#   "dve1_pool1" - subtract on VectorE, add on GPSIMD
#   "pe_add" - subtract on VectorE; add via PE identity-matmul accumulate
#              into PSUM, evacuated interleaved by ScalarE
VARIANT = "dve2"
LOAD_ENG = "sync"    # HWDGE ring for state loads ("sync"=SP, "scalar"=ACT)
STORE_ENG = "scalar"  # HWDGE ring for output stores
OUT_BUFS = 2
TMP_BUFS = 3
IO_BUFS = 3
PIPELINED_EMIT = False


def _phase_cos_sin(theta: np.ndarray):
    """Host-side computation of cos/sin of the ring phase (f64 -> f32)."""
    idx = np.arange(D, dtype=np.int64)
    shifts = (N_WIRES - 1) - np.arange(N_WIRES)
    bits = ((idx[:, None] >> shifts[None, :]) & 1).astype(np.float64)
    tgt_sign = 2.0 * np.roll(bits, -1, axis=1) - 1.0
    p = 0.5 * ((bits * tgt_sign) @ theta.astype(np.float64))
    return np.cos(p).astype(np.float32), np.sin(p).astype(np.float32)


def _split_multiwaits(nc):
    """Walrus in this container supports at most one sync-wait per
    instruction; hoist extra Tile-assigned waits onto single-wait NoOps."""
    import concourse.mybir as mybir

    for f in nc.m.functions:
        new_blocks = []
        for bb in f.blocks:
            insts = list(bb.instructions)
            if not any(
                i.sync_info is not None and len(i.sync_info.on_wait) > 1
                for i in insts
            ):
                new_blocks.append(bb)
                continue
            out = []
            for i in insts:
                si = i.sync_info
                if si is not None and len(si.on_wait) > 1:
                    waits = list(si.on_wait)
                    for k, w in enumerate(waits[:-1]):
                        out.append(
                            mybir.InstNoOp(
                                name=f"{i.name}-sw{k}",
                                engine=i.engine,
                                bass_nofuse=True,
                                sync_info=mybir.SyncInfo(on_wait=[w], on_update=[]),
                            )
                        )
                    i.sync_info = mybir.SyncInfo(
                        on_wait=[waits[-1]], on_update=list(si.on_update)
                    )
                out.append(i)
            new_blocks.append(mybir.BasicBlock(name=bb.name, instructions=out))
        f.blocks = new_blocks


def _build_nc(loop_n=None):
    """Build the per-core Bass program.

    loop_n: if set, wrap the whole body in a runtime For_i loop executing it
    loop_n times (benchmarking only — output is idempotent).
    """
    import contextlib

    import concourse.bass as bass
    import concourse.mybir as mybir
    from concourse.tile import TileContext

    nc = bass.Bass()
    f32 = mybir.dt.float32

    sr_d = nc.declare_dram_parameter("state_real", [ROWS, D], f32, isOutput=False)
    si_d = nc.declare_dram_parameter("state_imag", [ROWS, D], f32, isOutput=False)
    cs_d = nc.declare_dram_parameter("cs", [2, D], f32, isOutput=False)
    ones_d = nc.declare_dram_parameter("ones", [1, 128], f32, isOutput=False)
    eye_d = nc.declare_dram_parameter("eye", [128, 128], f32, isOutput=False)
    out_d = nc.declare_dram_parameter("out", [ROWS, 2 * D], f32, isOutput=True)

    with TileContext(nc) as tc:
        with (
            tc.tile_pool(name="const", bufs=1) as const_pool,
            tc.tile_pool(name="coef", bufs=2) as coef_pool,
            tc.tile_pool(name="io", bufs=IO_BUFS) as io_pool,
            tc.tile_pool(name="tmp", bufs=TMP_BUFS) as tmp_pool,
            tc.tile_pool(name="psum", bufs=2, space="PSUM") as psum_pool,
        ):
            ones_t = const_pool.tile([1, 128], f32)
            nc.sync.dma_start(out=ones_t, in_=ones_d[:, :])
            eye_t = None
            if VARIANT == "pe_add":
                eye_t = const_pool.tile([128, 128], f32)
                nc.sync.dma_start(out=eye_t, in_=eye_d[:, :])

            loop_cm = (
                tc.For_i(0, loop_n, 1) if loop_n else contextlib.nullcontext()
            )
            with loop_cm:
                _emit_body(nc, tc, coef_pool, io_pool, tmp_pool, psum_pool,
                           ones_t, eye_t, sr_d, si_d, cs_d, out_d, f32)

    _split_multiwaits(nc)
    return nc


def _emit_body(nc, tc, coef_pool, io_pool, tmp_pool, psum_pool,
               ones_t, eye_t, sr_d, si_d, cs_d, out_d, f32):
            coefs = {}
            plan = []  # (kind, arg) emission schedule: broadcast ci+1 ahead
            if PIPELINED_EMIT:
                for ci in range(N_CHUNK):
                    plan.append(("bcast", ci))
                    if ci > 0:
                        plan.append(("rgs", ci - 1))
                plan.append(("rgs", N_CHUNK - 1))
            else:
                for ci in range(N_CHUNK):
                    plan.append(("bcast", ci))
                    plan.append(("rgs", ci))
            for kind, ci in plan:
                d0 = ci * C
                if kind == "rgs":
                    cb, sb = coefs.pop(ci)
                    _emit_rgs(nc, io_pool, tmp_pool, psum_pool, eye_t,
                              sr_d, si_d, out_d, f32, d0, cb, sb)
                    continue
                c_t = coef_pool.tile([1, C], f32, tag="c_row")
                s_t = coef_pool.tile([1, C], f32, tag="s_row")
                nc.sync.dma_start(out=c_t, in_=cs_d[0:1, d0 : d0 + C])
                nc.sync.dma_start(out=s_t, in_=cs_d[1:2, d0 : d0 + C])
                cb = coef_pool.tile([128, C], f32, tag="cb")
                sb = coef_pool.tile([128, C], f32, tag="sb")
                for j in range(0, C, MM_N):
                    pc = psum_pool.tile([128, MM_N], f32, tag="pc")
                    nc.tensor.matmul(
                        pc, ones_t, c_t[:, j : j + MM_N], start=True, stop=True
                    )
                    nc.scalar.copy(out=cb[:, j : j + MM_N], in_=pc)
                    ps = psum_pool.tile([128, MM_N], f32, tag="ps")
                    nc.tensor.matmul(
                        ps, ones_t, s_t[:, j : j + MM_N], start=True, stop=True
                    )
                    nc.scalar.copy(out=sb[:, j : j + MM_N], in_=ps)

                coefs[ci] = (cb, sb)


def _emit_rgs(nc, io_pool, tmp_pool, psum_pool, eye_t,
              sr_d, si_d, out_d, f32, d0, cb, sb):
                for rg in range(ROWS // RG):
                    r0 = rg * RG
                    sr_t = io_pool.tile([RG, C], f32, tag="sr")
                    si_t = io_pool.tile([RG, C], f32, tag="si")
                    load_eng = getattr(nc, LOAD_ENG)
                    load_eng.dma_start(out=sr_t, in_=sr_d[r0 : r0 + RG, d0 : d0 + C])
                    load_eng.dma_start(out=si_t, in_=si_d[r0 : r0 + RG, d0 : d0 + C])

                    out_t = io_pool.tile([RG, 2 * C], f32, tag="out", bufs=OUT_BUFS)
                    sub_eng = nc.gpsimd if VARIANT == "pool2" else nc.vector
                    add_eng = nc.vector if VARIANT == "dve2" else nc.gpsimd
                    m1 = tmp_pool.tile([RG, C], f32, tag="m1")
                    m2 = tmp_pool.tile([RG, C], f32, tag="m2")
                    nc.vector.tensor_mul(out=m1, in0=sr_t, in1=cb)
                    nc.vector.tensor_mul(out=m2, in0=si_t, in1=sb)
                    sub_dst = (
                        out_t[:, 0:C] if VARIANT == "dve2_cont"
                        else out_t[:, 0 : 2 * C : 2]
                    )
                    sub_eng.tensor_sub(out=sub_dst, in0=m1, in1=m2)
                    m3 = tmp_pool.tile([RG, C], f32, tag="m1")
                    m4 = tmp_pool.tile([RG, C], f32, tag="m2")
                    nc.vector.tensor_mul(out=m3, in0=sr_t, in1=sb)
                    nc.vector.tensor_mul(out=m4, in0=si_t, in1=cb)
                    if VARIANT == "pool_cont":
                        # Pool adds contiguously; ACT interleaves into out_t
                        oi = tmp_pool.tile([RG, C], f32, tag="oi", bufs=2)
                        nc.gpsimd.tensor_add(out=oi, in0=m3, in1=m4)
                        nc.scalar.copy(out=out_t[:, 1 : 2 * C : 2], in_=oi)
                    elif VARIANT == "dve2_cont":
                        # DIAGNOSTIC ONLY (wrong layout): contiguous adds
                        nc.vector.tensor_add(out=out_t[:, C : 2 * C], in0=m3, in1=m4)
                    elif VARIANT == "pe_add":
                        for j in range(0, C, 2 * MM_N):
                            pj = psum_pool.tile([128, 2 * MM_N], f32, tag="padd")
                            for h in (0, MM_N):
                                nc.tensor.matmul(
                                    pj[:, h : h + MM_N], eye_t,
                                    m3[:, j + h : j + h + MM_N],
                                    start=True, stop=False,
                                )
                                nc.tensor.matmul(
                                    pj[:, h : h + MM_N], eye_t,
                                    m4[:, j + h : j + h + MM_N],
                                    start=False, stop=True,
                                )
                            nc.scalar.copy(
                                out=out_t[:, 2 * j + 1 : 2 * (j + 2 * MM_N) : 2],
                                in_=pj,
                            )
                    else:
                        add_eng.tensor_add(
                            out=out_t[:, 1 : 2 * C : 2], in0=m3, in1=m4
                        )
                    getattr(nc, STORE_ENG).dma_start(
                        out=out_d[r0 : r0 + RG, 2 * d0 : 2 * d0 + 2 * C], in_=out_t
                    )


def _get_nc():
    global _CACHED_NC
    if _CACHED_NC is None:
        _CACHED_NC = _build_nc()
    return _CACHED_NC


def _make_in_maps(state_real, state_imag, theta):
    state_real = np.ascontiguousarray(np.asarray(state_real, dtype=np.float32))
    state_imag = np.ascontiguousarray(np.asarray(state_imag, dtype=np.float32))
    theta = np.asarray(theta, dtype=np.float32)
    c, s = _phase_cos_sin(theta)
    cs = np.ascontiguousarray(np.stack([c, s], axis=0))
    ones = np.ones((1, 128), dtype=np.float32)
    eye = np.eye(128, dtype=np.float32)
    in_maps = []
    for k in range(N_CORES):
        r0 = k * ROWS
        in_maps.append(
            {
                "state_real": state_real[r0 : r0 + ROWS],
                "state_imag": state_imag[r0 : r0 + ROWS],
                "cs": cs,
                "ones": ones,
                "eye": eye,
            }
        )
    return in_maps


def kernel(state_real, state_imag, theta):
    from concourse.bass_utils import run_bass_kernel_spmd

    nc = _get_nc()
    in_maps = _make_in_maps(state_real, state_imag, theta)
    res = run_bass_kernel_spmd(nc, in_maps, list(range(N_CORES)))
    out = np.empty((B, D, 2), dtype=np.float32)
    for k in range(N_CORES):
        out[k * ROWS : (k + 1) * ROWS] = res.results[k]["out"].reshape(ROWS, D, 2)
    return out


# revision 22
# speedup vs baseline: 332.3813x; 1.0393x over previous
"""CRZ-ring fused diagonal phase rotation on 8 Trainium2 NeuronCores.

Computation (reference):
    p[d]  = 0.5 * sum_i bits[d,i] * (2*bits[d,(i+1)%14] - 1) * theta[i]
    out_r = state_real * cos(p) - state_imag * sin(p)
    out_i = state_real * sin(p) + state_imag * cos(p)
    out   = stack([out_r, out_i], axis=-1)          # [B, D, 2] f32

Strategy:
  - cos(p)/sin(p) ([D] vectors) are computed on the host (tiny: 16K elems).
  - Batch dim (2048) is sharded across 8 cores (256 rows each).
  - Per core, per chunk of C=2048 columns: cos/sin rows are broadcast to
    [128, C] tiles via a K=1 PE matmul (ones ⊗ row) + ScalarE PSUM→SBUF copy.
    VectorE does the 4 products, GPSIMD does the 2 interleaving add/subs
    writing the [128, 2C] interleaved output tile, HWDGE DMA does all I/O.
"""

import numpy as np

B = 2048
D = 16384
N_WIRES = 14
N_CORES = 8
ROWS = B // N_CORES      # 256 batch rows per core
RG = 128                 # rows per partition group
C = 2048                 # d-chunk per tile
N_CHUNK = D // C
MM_N = 512               # matmul moving free dim (one PSUM bank)

_CACHED_NC = None

# Engine-split variant for the two interleaving add/subs (perf tuning):
#   "pool2"  - both on GPSIMD
#   "dve2"   - both on VectorE
#   "dve1_pool1" - subtract on VectorE, add on GPSIMD
#   "pe_add" - subtract on VectorE; add via PE identity-matmul accumulate
#              into PSUM, evacuated interleaved by ScalarE
VARIANT = "dve2"
LOAD_ENG = "sync"    # HWDGE ring for state loads ("sync"=SP, "scalar"=ACT)
STORE_ENG = "scalar"  # HWDGE ring for output stores
OUT_BUFS = 2
TMP_BUFS = 3
IO_BUFS = 3
PIPELINED_EMIT = False


def _phase_cos_sin(theta: np.ndarray):
    """Host-side computation of cos/sin of the ring phase (f64 -> f32)."""
    idx = np.arange(D, dtype=np.int64)
    shifts = (N_WIRES - 1) - np.arange(N_WIRES)
    bits = ((idx[:, None] >> shifts[None, :]) & 1).astype(np.float64)
    tgt_sign = 2.0 * np.roll(bits, -1, axis=1) - 1.0
    p = 0.5 * ((bits * tgt_sign) @ theta.astype(np.float64))
    return np.cos(p).astype(np.float32), np.sin(p).astype(np.float32)


def _split_multiwaits(nc):
    """Walrus in this container supports at most one sync-wait per
    instruction; hoist extra Tile-assigned waits onto single-wait NoOps."""
    import concourse.mybir as mybir

    for f in nc.m.functions:
        new_blocks = []
        for bb in f.blocks:
            insts = list(bb.instructions)
            if not any(
                i.sync_info is not None and len(i.sync_info.on_wait) > 1
                for i in insts
            ):
                new_blocks.append(bb)
                continue
            out = []
            for i in insts:
                si = i.sync_info
                if si is not None and len(si.on_wait) > 1:
                    waits = list(si.on_wait)
                    for k, w in enumerate(waits[:-1]):
                        out.append(
                            mybir.InstNoOp(
                                name=f"{i.name}-sw{k}",
                                engine=i.engine,
                                bass_nofuse=True,
                                sync_info=mybir.SyncInfo(on_wait=[w], on_update=[]),
                            )
                        )
                    i.sync_info = mybir.SyncInfo(
                        on_wait=[waits[-1]], on_update=list(si.on_update)
                    )
                out.append(i)
            new_blocks.append(mybir.BasicBlock(name=bb.name, instructions=out))
        f.blocks = new_blocks


def _build_nc(loop_n=None):
    """Build the per-core Bass program.

    loop_n: if set, wrap the whole body in a runtime For_i loop executing it
    loop_n times (benchmarking only — output is idempotent).
    """
    import contextlib

    import concourse.bass as bass
    import concourse.mybir as mybir
    from concourse.tile import TileContext

    nc = bass.Bass()
    f32 = mybir.dt.float32

    sr_d = nc.declare_dram_parameter("state_real", [ROWS, D], f32, isOutput=False)
    si_d = nc.declare_dram_parameter("state_imag", [ROWS, D], f32, isOutput=False)
    cs_d = nc.declare_dram_parameter("cs", [2, D], f32, isOutput=False)
    ones_d = nc.declare_dram_parameter("ones", [1, 128], f32, isOutput=False)
    eye_d = nc.declare_dram_parameter("eye", [128, 128], f32, isOutput=False)
    out_d = nc.declare_dram_parameter("out", [ROWS, 2 * D], f32, isOutput=True)

    with TileContext(nc) as tc:
        with (
            tc.tile_pool(name="const", bufs=1) as const_pool,
            tc.tile_pool(name="coef", bufs=2) as coef_pool,
            tc.tile_pool(name="io", bufs=IO_BUFS) as io_pool,
            tc.tile_pool(name="tmp", bufs=TMP_BUFS) as tmp_pool,
            tc.tile_pool(name="psum", bufs=2, space="PSUM") as psum_pool,
        ):
            ones_t = const_pool.tile([1, 128], f32)
            nc.sync.dma_start(out=ones_t, in_=ones_d[:, :])
            eye_t = None
            if VARIANT == "pe_add":
                eye_t = const_pool.tile([128, 128], f32)
                nc.sync.dma_start(out=eye_t, in_=eye_d[:, :])

            loop_cm = (
                tc.For_i(0, loop_n, 1) if loop_n else contextlib.nullcontext()
            )
            with loop_cm:
                _emit_body(nc, tc, coef_pool, io_pool, tmp_pool, psum_pool,
                           ones_t, eye_t, sr_d, si_d, cs_d, out_d, f32)

    _split_multiwaits(nc)
    return nc


def _emit_body(nc, tc, coef_pool, io_pool, tmp_pool, psum_pool,
               ones_t, eye_t, sr_d, si_d, cs_d, out_d, f32):
            coefs = {}
            plan = []  # (kind, arg) emission schedule: broadcast ci+1 ahead
            if PIPELINED_EMIT:
                for ci in range(N_CHUNK):
                    plan.append(("bcast", ci))
                    if ci > 0:
                        plan.append(("rgs", ci - 1))
                plan.append(("rgs", N_CHUNK - 1))
            else:
                for ci in range(N_CHUNK):
                    plan.append(("bcast", ci))
                    plan.append(("rgs", ci))
            for kind, ci in plan:
                d0 = ci * C
                if kind == "rgs":
                    cb, sb = coefs.pop(ci)
                    _emit_rgs(nc, io_pool, tmp_pool, psum_pool, eye_t,
                              sr_d, si_d, out_d, f32, d0, cb, sb)
                    continue
                c_t = coef_pool.tile([1, C], f32, tag="c_row")
                s_t = coef_pool.tile([1, C], f32, tag="s_row")
                nc.sync.dma_start(out=c_t, in_=cs_d[0:1, d0 : d0 + C])
                nc.sync.dma_start(out=s_t, in_=cs_d[1:2, d0 : d0 + C])
                cb = coef_pool.tile([128, C], f32, tag="cb")
                sb = coef_pool.tile([128, C], f32, tag="sb")
                for j in range(0, C, MM_N):
                    pc = psum_pool.tile([128, MM_N], f32, tag="pc")
                    nc.tensor.matmul(
                        pc, ones_t, c_t[:, j : j + MM_N], start=True, stop=True
                    )
                    nc.scalar.copy(out=cb[:, j : j + MM_N], in_=pc)
                    ps = psum_pool.tile([128, MM_N], f32, tag="ps")
                    nc.tensor.matmul(
                        ps, ones_t, s_t[:, j : j + MM_N], start=True, stop=True
                    )
                    nc.scalar.copy(out=sb[:, j : j + MM_N], in_=ps)

                coefs[ci] = (cb, sb)


def _emit_rgs(nc, io_pool, tmp_pool, psum_pool, eye_t,
              sr_d, si_d, out_d, f32, d0, cb, sb):
                for rg in range(ROWS // RG):
                    r0 = rg * RG
                    sr_t = io_pool.tile([RG, C], f32, tag="sr")
                    si_t = io_pool.tile([RG, C], f32, tag="si")
                    load_eng = getattr(nc, LOAD_ENG)
                    load_eng.dma_start(out=sr_t, in_=sr_d[r0 : r0 + RG, d0 : d0 + C])
                    load_eng.dma_start(out=si_t, in_=si_d[r0 : r0 + RG, d0 : d0 + C])

                    out_t = io_pool.tile([RG, 2 * C], f32, tag="out", bufs=OUT_BUFS)
                    sub_eng = nc.gpsimd if VARIANT == "pool2" else nc.vector
                    add_eng = nc.vector if VARIANT == "dve2" else nc.gpsimd
                    m1 = tmp_pool.tile([RG, C], f32, tag="m1")
                    m2 = tmp_pool.tile([RG, C], f32, tag="m2")
                    nc.vector.tensor_mul(out=m1, in0=sr_t, in1=cb)
                    nc.vector.tensor_mul(out=m2, in0=si_t, in1=sb)
                    sub_dst = (
                        out_t[:, 0:C] if VARIANT == "dve2_cont"
                        else out_t[:, 0 : 2 * C : 2]
                    )
                    sub_eng.tensor_sub(out=sub_dst, in0=m1, in1=m2)
                    m3 = tmp_pool.tile([RG, C], f32, tag="m1")
                    m4 = tmp_pool.tile([RG, C], f32, tag="m2")
                    nc.vector.tensor_mul(out=m3, in0=sr_t, in1=sb)
                    nc.vector.tensor_mul(out=m4, in0=si_t, in1=cb)
                    if VARIANT == "pool_cont":
                        # Pool adds contiguously; ACT interleaves into out_t
                        oi = tmp_pool.tile([RG, C], f32, tag="oi", bufs=2)
                        nc.gpsimd.tensor_add(out=oi, in0=m3, in1=m4)
                        nc.scalar.copy(out=out_t[:, 1 : 2 * C : 2], in_=oi)
                    elif VARIANT == "dve2_cont":
                        # DIAGNOSTIC ONLY (wrong layout): contiguous adds
                        nc.vector.tensor_add(out=out_t[:, C : 2 * C], in0=m3, in1=m4)
                    elif VARIANT == "pe_add":
                        for j in range(0, C, 2 * MM_N):
                            pj = psum_pool.tile([128, 2 * MM_N], f32, tag="padd")
                            for h in (0, MM_N):
                                nc.tensor.matmul(
                                    pj[:, h : h + MM_N], eye_t,
                                    m3[:, j + h : j + h + MM_N],
                                    start=True, stop=False,
                                )
                                nc.tensor.matmul(
                                    pj[:, h : h + MM_N], eye_t,
                                    m4[:, j + h : j + h + MM_N],
                                    start=False, stop=True,
                                )
                            nc.scalar.copy(
                                out=out_t[:, 2 * j + 1 : 2 * (j + 2 * MM_N) : 2],
                                in_=pj,
                            )
                    else:
                        add_eng.tensor_add(
                            out=out_t[:, 1 : 2 * C : 2], in0=m3, in1=m4
                        )
                    getattr(nc, STORE_ENG).dma_start(
                        out=out_d[r0 : r0 + RG, 2 * d0 : 2 * d0 + 2 * C], in_=out_t
                    )


def _get_nc():
    global _CACHED_NC
    if _CACHED_NC is None:
        _CACHED_NC = _build_nc()
    return _CACHED_NC


def _make_in_maps(state_real, state_imag, theta):
    state_real = np.ascontiguousarray(np.asarray(state_real, dtype=np.float32))
    state_imag = np.ascontiguousarray(np.asarray(state_imag, dtype=np.float32))
    theta = np.asarray(theta, dtype=np.float32)
    c, s = _phase_cos_sin(theta)
    cs = np.ascontiguousarray(np.stack([c, s], axis=0))
    ones = np.ones((1, 128), dtype=np.float32)
    eye = np.eye(128, dtype=np.float32)
    in_maps = []
    for k in range(N_CORES):
        r0 = k * ROWS
        in_maps.append(
            {
                "state_real": state_real[r0 : r0 + ROWS],
                "state_imag": state_imag[r0 : r0 + ROWS],
                "cs": cs,
                "ones": ones,
                "eye": eye,
            }
        )
    return in_maps


def kernel(state_real, state_imag, theta):
    from concourse.bass_utils import run_bass_kernel_spmd

    nc = _get_nc()
    in_maps = _make_in_maps(state_real, state_imag, theta)
    try:
        res = run_bass_kernel_spmd(nc, in_maps, list(range(N_CORES)))
    except Exception:
        res = run_bass_kernel_spmd(nc, in_maps, list(range(N_CORES)))
    out = np.empty((B, D, 2), dtype=np.float32)
    for k in range(N_CORES):
        out[k * ROWS : (k + 1) * ROWS] = res.results[k]["out"].reshape(ROWS, D, 2)
    return out
